# revision 1
# baseline (speedup 1.0000x reference)
"""Trainium2 Bass kernel for nn_Attn_30683246362810 (block-diagonal attention).

Sharding: data-parallel over the 8 equal-length packed sequences
(cu_seqlens = arange*1024) -- core i processes batch i independently,
no collectives.

Per-core pipeline (feature-major activation layout [feature, token]):
  xT -> rmsnorm (sum(x^2) via ones-matmul over partitions) -> hT
     -> QKV matmuls (weights pre-transposed + rotary-deinterleave-permuted
        on the host) -> rotary on DVE -> per-head GQA attention:
        S^T = kT.T @ qT (4-way row-tiled k=32 matmuls, one head per PSUM
        bank), exp on ScalarE (no max subtraction -- scores are O(1) here),
        P@V with a ones-row-augmented v so the softmax denominator falls out
        in PSUM row 64, normalize + sigmoid-gate -> out projection back to
        natural [token, dim] layout.

Projection matmuls run as float32r (full PE rate; true fp32 is 4 cyc/row).
Attention operands (q,k,P,v) are bf16: same PE rate, half SBUF.
"""

import numpy as np

import concourse.bass as bass
import concourse.mybir as mybir
from concourse.tile import TileContext
from concourse.vector_clock import ScopedClock, VectorClock
from concourse.tile_sem_assignment import N_PROCS
from concourse.bass_utils import run_bass_kernel_spmd

F32 = mybir.dt.float32
F32R = mybir.dt.float32r
BF16 = mybir.dt.bfloat16
AF = mybir.ActivationFunctionType
ALU = mybir.AluOpType

N_CORES = 8
T = 1024          # tokens per core (one packed sequence)
D = 1024          # model dim
QH = 16           # query heads
KVH = 4           # kv heads
HD = 64           # head dim
F = HD // 2       # 32 rotary freqs
EPS = 1e-6
SCALE = 1.0 / np.sqrt(HD)
NT = T // 128     # 8 token tiles
ND = D // 128     # 8 dim tiles
NC2 = 2           # token chunks of 512 (fp32 matmul moving-dim max)
CH = 512


class _TC(TileContext):
    """TileContext whose final drain splits its sem waits into 1-wait nops
    (this walrus build rejects >1 sync wait per instruction)."""

    def _drain_and_barrier(self, tick_clock, wait_clock):
        gc = tick_clock.global_clock
        for p in range(N_PROCS):
            t = gc[p]
            if t > 0:
                one = VectorClock([t if q == p else 0 for q in range(N_PROCS)])
                nop = self.nc.sync.add_instruction(
                    mybir.InstNoOp(name=f"I-{self.nc.next_id()}",
                                   engine=mybir.EngineType.SP, bass_nofuse=True))
                wait_clock.add_sem_waits(nop.ins, ScopedClock({None: one}))
        self.nc.sync.drain()
        self.nc.all_engine_barrier()
        assert self.sems is not None
        popped = self.nc._tile_sem_poison_stack.pop()
        assert popped is self._sem_poison
        self.nc.clear_and_free_semaphores(list(self.sems.allocated().values()))
        self.nc.all_engine_barrier()


def _split_multiwaits(nc):
    """Hoist extra sync waits onto preceding same-engine NoOps (1-wait limit)."""
    for f in nc.m.functions:
        for bb in f.blocks:
            insts = list(bb.instructions)
            if not any(i.sync_info is not None and len(i.sync_info.on_wait) > 1
                       for i in insts):
                continue
            new = []
            for i in insts:
                si = i.sync_info
                if si is not None and len(si.on_wait) > 1:
                    waits = list(si.on_wait)
                    for w in waits[:-1]:
                        new.append(mybir.InstNoOp(
                            name=f"I-{nc.next_id()}", engine=i.engine,
                            bass_nofuse=True,
                            sync_info=mybir.SyncInfo(on_wait=[w], on_update=[])))
                    i.sync_info = mybir.SyncInfo(on_wait=[waits[-1]],
                                                 on_update=list(si.on_update))
                new.append(i)
            bb.instructions = new


def _bcast_ap(src_ap, n):
    """AP reading src_ap's single partition replicated across n partitions."""
    return bass.AP(tensor=src_ap.tensor, offset=src_ap.offset,
                   ap=[[0, n]] + [list(d) for d in src_ap.ap[1:]])


def _rep_ap(src_ap, reps):
    """AP replicating src_ap's partition block `reps` times."""
    return bass.AP(tensor=src_ap.tensor, offset=src_ap.offset,
                   ap=[[0, reps]] + [list(d) for d in src_ap.ap])


def r32(ap):
    return ap.bitcast(F32R)


def build_nc(debug=False, split=True, reps=1):
    nc = bass.Bass("TRN2", dynamic_dma_scratch_size=8192)

    xT_d = nc.dram_tensor("xT", [128, ND, T], F32, kind="ExternalInput")
    freqsT_d = nc.dram_tensor("freqsT", [F, T], F32, kind="ExternalInput")
    g_d = nc.dram_tensor("g", [D], F32, kind="ExternalInput")
    wqkvT_d = nc.dram_tensor("wqkvT", [20, 128, ND, 128], F32R, kind="ExternalInput")
    woutT_d = nc.dram_tensor("woutT", [128, ND, D], F32R, kind="ExternalInput")
    out_d = nc.dram_tensor("out", [T, D], F32, kind="ExternalOutput")
    rstd_dr = nc.dram_tensor("rstd_scratch", [T], F32, kind="Internal")
    rs_dr = nc.dram_tensor("rs_scratch", [8, 4 * CH], F32, kind="Internal")
    dbg = {}
    if debug:
        dbg["hT"] = nc.dram_tensor("dbg_hT", [D, T], F32, kind="ExternalOutput")
        dbg["q1"] = nc.dram_tensor("dbg_q1", [512, T], F32, kind="ExternalOutput")
        dbg["og"] = nc.dram_tensor("dbg_og", [D, T], F32, kind="ExternalOutput")

    with _TC(nc) as tc:
        with (
            tc.tile_pool(name="per", bufs=1) as per,            # persistent (bufs=1/tag)
            tc.tile_pool(name="qkx", bufs=3) as qkx,    # pre-rotary q/k tiles
            tc.tile_pool(name="wstr", bufs=2) as wstr,  # W_qkv streaming
            tc.tile_pool(name="scr", bufs=4) as scr,    # rotary scratch
            tc.tile_pool(name="sq", bufs=2) as sqp,     # x^2 scratch
            tc.tile_pool(name="krp", bufs=2) as krp,    # replicated k tiles
            tc.tile_pool(name="pexp", bufs=2) as pexp,  # exp(S^T) tiles
            tc.tile_pool(name="nrm", bufs=1) as nrm,
            tc.tile_pool(name="oup", bufs=2) as oup,    # recip/broadcast tiles
            tc.tile_pool(name="ob", bufs=1) as ob,      # output staging
            tc.tile_pool(name="psp", bufs=1, space="PSUM") as psp,
        ):
            def _emit(rep):
                debug_r = debug and rep == 0
                _pp = [0]

                def mm_ps(shape):
                    _pp[0] ^= 1
                    return psp.tile(shape, F32, tag=("mm1" if _pp[0] else "mm2"),
                                    name="mmps")

                # ---------------- phase A: load + rmsnorm ----------------
                xT = per.tile([128, ND, T], F32, tag="xT")
                nc.sync.dma_start(out=xT[:], in_=xT_d[:, :, :])
                g_sb = per.tile([128, ND], F32, tag="g")
                nc.sync.dma_start(out=g_sb[:], in_=g_d[:].rearrange(
                    "(j p) -> p j", p=128))

                ones_f = per.tile([128, 1], F32, tag="ones_f")
                nc.vector.memset(ones_f[:], 1.0)
                ones_col = per.tile([128, 1], F32R, tag="ones")
                nc.vector.tensor_copy(ones_col[:], ones_f[:])
                eps_sb = per.tile([1, 1], F32, tag="eps")
                nc.vector.memset(eps_sb[:], EPS)

                ssq = per.tile([1, T], F32, tag="ssq")
                for c in range(NC2):
                    sl = slice(c * CH, (c + 1) * CH)
                    ps_s = mm_ps([1, CH])
                    for j in range(ND):
                        xsq = sqp.tile([128, CH], F32R, tag="xsq")
                        nc.scalar.activation(out=xsq[:], in_=xT[:, j, sl],
                                             func=AF.Square)
                        nc.tensor.matmul(ps_s[:, :], ones_col[:], xsq[:],
                                         start=(j == 0), stop=(j == ND - 1))
                    nc.vector.tensor_copy(ssq[0:1, sl], ps_s[:, :])

                rstd = per.tile([1, T], F32, tag="rstd")
                nc.scalar.activation(out=rstd[:], in_=ssq[:], func=AF.Sqrt,
                                     bias=eps_sb[:], scale=1.0 / D)
                nc.vector.reciprocal(rstd[:], rstd[:])
                rstd_b = per.tile([128, T], F32, tag="rstd_b")
                nc.sync.dma_start(out=rstd_dr[:], in_=rstd[0:1, :])
                nc.sync.dma_start(out=rstd_b[:], in_=_rep_ap(rstd_dr[:], 128))

                hT = per.tile([128, ND, T], F32R, tag="hT")
                for j in range(ND):
                    nc.vector.scalar_tensor_tensor(
                        out=hT[:, j, :], in0=xT[:, j, :], scalar=g_sb[:, j:j + 1],
                        in1=rstd_b[:, :], op0=ALU.mult, op1=ALU.mult)
                if debug_r:
                    nc.sync.dma_start(
                        out=dbg["hT"][:, :].rearrange("(j p) t -> p j t", p=128),
                        in_=hT[:])

                # ------------- rotary cos/sin (fp32, [128, T] = 4x replicated) ----
                freqs128 = scr.tile([128, T], F32, tag="rot")
                nc.gpsimd.dma_start(out=freqs128[:],
                                    in_=_rep_ap(freqsT_d[:, :], 4))
                S4 = per.tile([128, T], BF16, tag="S4")
                C4 = per.tile([128, T], BF16, tag="C4")
                TWO_PI = float(2 * np.pi)

                def trig(dst, shift):
                    # dst = sin(freqs + shift); ACT Sin domain is [-pi, pi], so
                    # correct by -+2pi where (freqs + shift) leaves it (|arg|<3pi).
                    bias = per.tile([128, 1], F32, tag=f"bias{shift:.2f}",
                                    name="trig_bias")
                    nc.vector.memset(bias[:], float(shift))
                    a = scr.tile([128, T], F32, tag="rot", name="trig_a")
                    nc.vector.tensor_scalar(out=a[:], in0=freqs128[:],
                                            scalar1=float(np.pi - shift), scalar2=None,
                                            op0=ALU.is_ge)
                    b = scr.tile([128, T], F32, tag="rot", name="trig_b")
                    nc.vector.tensor_scalar(out=b[:], in0=freqs128[:],
                                            scalar1=float(-np.pi - shift), scalar2=None,
                                            op0=ALU.is_lt)
                    t1 = scr.tile([128, T], F32, tag="rot", name="trig_t1")
                    nc.vector.scalar_tensor_tensor(
                        out=t1[:], in0=a[:], scalar=-TWO_PI, in1=freqs128[:],
                        op0=ALU.mult, op1=ALU.add)
                    t2 = scr.tile([128, T], F32, tag="rot", name="trig_t2")
                    nc.vector.scalar_tensor_tensor(
                        out=t2[:], in0=b[:], scalar=TWO_PI, in1=t1[:],
                        op0=ALU.mult, op1=ALU.add)
                    nc.scalar.activation(out=dst, in_=t2[:], func=AF.Sin, bias=bias[:])

                trig(S4[:], 0.0)
                trig(C4[:], float(np.pi / 2))

                rq1 = per.tile([128, 4, T], BF16, tag="rq1")
                rq2 = per.tile([128, 4, T], BF16, tag="rq2")
                rk1 = per.tile([128, T], BF16, tag="rk1")
                rk2 = per.tile([128, T], BF16, tag="rk2")
                sg = per.tile([128, ND, T], BF16, tag="sg")

                def rotate(x1, x2, o1, o2):
                    m1 = scr.tile([128, T], F32, tag="rot")
                    m2 = scr.tile([128, T], F32, tag="rot")
                    nc.vector.tensor_mul(m1[:], x1, C4[:])
                    nc.vector.tensor_mul(m2[:], x2, S4[:])
                    nc.vector.tensor_sub(o1, m1[:], m2[:])
                    m3 = scr.tile([128, T], F32, tag="rot")
                    m4 = scr.tile([128, T], F32, tag="rot")
                    nc.vector.tensor_mul(m3[:], x1, S4[:])
                    nc.vector.tensor_mul(m4[:], x2, C4[:])
                    nc.vector.tensor_add(o2, m3[:], m4[:])

                # v first (needed by every attention group): natural
                # layout, ones-augmented (bf16): [128, NT, KVH, 65]
                v_aug = per.tile([128, NT, KVH, HD + 1], BF16, tag="v_aug")
                nc.vector.memset(v_aug[:, :, :, HD], 1.0)
                wv = per.tile([128, ND, 256], F32R, tag="wv")
                nc.sync.dma_start(out=wv[:, :, 0:128], in_=wqkvT_d[18])
                nc.sync.dma_start(out=wv[:, :, 128:256], in_=wqkvT_d[19])
                for tt in range(NT):
                    ps_v = mm_ps([128, KVH, HD])
                    for j in range(ND):
                        nc.tensor.matmul(ps_v[:, :, :],
                                         hT[:, j, tt * 128:(tt + 1) * 128],
                                         wv[:, j, :],
                                         start=(j == 0), stop=(j == ND - 1))
                    nc.vector.tensor_copy(v_aug[:, tt, :, 0:HD], ps_v[:, :, :])

                pre = {}

                def emit_qkv_tile(o):
                    wblk = wstr.tile([128, ND, 128], F32R, tag="wblk",
                                     name="wblk")
                    nc.sync.dma_start(out=wblk[:], in_=wqkvT_d[o])
                    dest = None if o < 8 or o >= 16 else sg[:, o - 8, :]
                    if dest is None:
                        t_pre = qkx.tile([128, T], BF16, tag="qk", name="qk")
                        pre[o] = t_pre
                        dest = t_pre[:]
                    for c in range(NC2):
                        sl = slice(c * CH, (c + 1) * CH)
                        ps_q = mm_ps([128, CH])
                        for j in range(ND):
                            nc.tensor.matmul(ps_q[:, :], wblk[:, j, :],
                                             hT[:, j, sl],
                                             start=(j == 0), stop=(j == ND - 1))
                        if 8 <= o < 16:
                            nc.scalar.activation(out=dest[:, sl], in_=ps_q[:, :],
                                                 func=AF.Sigmoid)
                        else:
                            nc.vector.tensor_copy(dest[:, sl], ps_q[:, :])

                def emit_attention(i):
                    krep1 = krp.tile([128, T], BF16, tag="krep1", name="krep1")
                    krep2 = krp.tile([128, T], BF16, tag="krep2", name="krep2")
                    for a in range(4):
                        nc.sync.dma_start(out=krep1[a * F:(a + 1) * F, :],
                                          in_=rk1[i * F:(i + 1) * F, :])
                        nc.sync.dma_start(out=krep2[a * F:(a + 1) * F, :],
                                          in_=rk2[i * F:(i + 1) * F, :])
                    for c in range(NC2):
                        tq = slice(c * CH, (c + 1) * CH)
                        o_ps = psp.tile([HD + 1, 4, CH], F32, tag="ov",
                                        name="o_ps")
                        for tk in range(NT):
                            tks = slice(tk * 128, (tk + 1) * 128)
                            for half in range(2):       # heads 2*half,2*half+1
                                s_ps = psp.tile([128, 2, CH], F32, tag="sc",
                                                name="s_ps")
                                for b2 in range(2):
                                    aa = 2 * half + b2
                                    nc.tensor.matmul(
                                        s_ps[:, b2, :],
                                        krep1[aa * F:(aa + 1) * F, tks],
                                        rq1[aa * F:(aa + 1) * F, i, tq],
                                        start=True, stop=False,
                                        tile_position=(aa * F, 0))
                                for b2 in range(2):
                                    aa = 2 * half + b2
                                    nc.tensor.matmul(
                                        s_ps[:, b2, :],
                                        krep2[aa * F:(aa + 1) * F, tks],
                                        rq2[aa * F:(aa + 1) * F, i, tq],
                                        start=False, stop=True,
                                        tile_position=(aa * F, 0))
                                p_sb = pexp.tile([128, 2, CH], BF16,
                                                 tag="p_sb", name="p_sb")
                                nc.scalar.activation(out=p_sb[:], in_=s_ps[:],
                                                     func=AF.Exp,
                                                     scale=float(SCALE))
                                for b2 in range(2):
                                    aa = 2 * half + b2
                                    nc.tensor.matmul(
                                        o_ps[:, aa, :],
                                        v_aug[:, tk, i, :],
                                        p_sb[:, b2, :],
                                        start=(tk == 0), stop=(tk == NT - 1))
                        # evacuate PV psum fast so the next group can start
                        ou = oup.tile([HD + 1, 4, CH], F32, tag="ou", name="ou")
                        nc.scalar.copy(ou[:], o_ps[:])
                        rs = nrm.tile([HD + 1, 4, CH], F32, tag="rs", name="rs")
                        nc.vector.reciprocal(rs[HD:HD + 1, :, :],
                                             ou[HD:HD + 1, :, :])
                        R = nrm.tile([HD, 4, CH], F32, tag="R", name="R")
                        nc.sync.dma_start(out=rs_dr[2 * i + c, :],
                                          in_=rs[HD:HD + 1, :, :])
                        nc.sync.dma_start(
                            out=R[:, :, :],
                            in_=_rep_ap(rs_dr[2 * i + c, :].rearrange(
                                "(a t) -> a t", a=4), HD))
                        for a in range(4):
                            h = 4 * i + a
                            rows = slice((h % 2) * HD, (h % 2) * HD + HD)
                            dst = ogT[rows, h // 2, tq]
                            nc.vector.tensor_mul(dst, ou[0:HD, a, :],
                                                 R[:, a, :])
                            nc.vector.tensor_mul(dst, dst,
                                                 sg[rows, h // 2, tq])

                ogT = per.tile([128, ND, T], F32R, tag="xT")  # reuse xT slot
                emit_qkv_tile(16)
                emit_qkv_tile(17)
                rotate(pre[16][:], pre[17][:], rk1[:], rk2[:])
                del pre[16], pre[17]
                for i in range(4):
                    emit_qkv_tile(i)
                    emit_qkv_tile(4 + i)
                    rotate(pre[i][:], pre[4 + i][:], rq1[:, i, :], rq2[:, i, :])
                    del pre[i], pre[4 + i]
                    emit_qkv_tile(8 + 2 * i)
                    emit_qkv_tile(9 + 2 * i)
                    emit_attention(i)

                if debug_r:
                    nc.sync.dma_start(
                        out=dbg["q1"][:, :].rearrange("(j p) t -> p j t", p=128),
                        in_=rq1[:])
                if debug_r:
                    nc.sync.dma_start(
                        out=dbg["og"][:, :].rearrange("(j p) t -> p j t", p=128),
                        in_=ogT[:])

                # ---------------- phase D: out projection ----------------
                wout_sb = per.tile([128, ND, D], F32R, tag="hT")  # reuse hT's slot
                nc.sync.dma_start(out=wout_sb[:], in_=woutT_d[:, :, :])
                for tt in range(NT):
                    tts = slice(tt * 128, (tt + 1) * 128)
                    for c in range(NC2):
                        sl = slice(c * CH, (c + 1) * CH)
                        ps_o = mm_ps([128, CH])
                        for j in range(ND):
                            nc.tensor.matmul(
                                ps_o[:, :], ogT[:, j, tts],
                                wout_sb[:, j, sl],
                                start=(j == 0), stop=(j == ND - 1))
                        o_sb = ob.tile([128, CH], F32, tag="o_sb")
                        nc.vector.tensor_copy(o_sb[:], ps_o[:, :])
                        nc.sync.dma_start(out=out_d[tts, sl], in_=o_sb[:])

            for _rep in range(reps):
                _emit(_rep)

    if split:
        _split_multiwaits(nc)
    return nc


def _host_prep(x, freqs, g, W_qkv, W_out):
    # W_qkv^T column layout (o): [q_x1 512 | q_x2 512 | gate 1024 |
    #                             k_x1 128 | k_x2 128 | v 256]
    perm = []
    for h in range(QH):
        perm += [h * HD + 2 * f for f in range(F)]
    for h in range(QH):
        perm += [h * HD + 2 * f + 1 for f in range(F)]
    perm += list(range(D, 2 * D))
    for gg in range(KVH):
        perm += [2 * D + gg * HD + 2 * f for f in range(F)]
    for gg in range(KVH):
        perm += [2 * D + gg * HD + 2 * f + 1 for f in range(F)]
    perm += list(range(2 * D + 256, 2 * D + 512))
    wqkvT = np.ascontiguousarray(W_qkv[perm].T, dtype=np.float32)
    # device-tile order: [o_tile, p, j, c] with d = j*128+p, o = o_tile*128+c
    wqkvT = np.ascontiguousarray(
        wqkvT.reshape(8, 128, 20, 128).transpose(2, 1, 0, 3))
    woutT = np.ascontiguousarray(
        W_out.T.reshape(8, 128, 1024).transpose(1, 0, 2).astype(np.float32))
    g = np.ascontiguousarray(g, dtype=np.float32)
    in_maps = []
    for ci in range(N_CORES):
        sl = slice(ci * T, (ci + 1) * T)
        in_maps.append({
            "xT": np.ascontiguousarray(
                x[sl].T.reshape(8, 128, 1024).transpose(1, 0, 2).astype(
                    np.float32)),
            "freqsT": np.ascontiguousarray(freqs[sl].T, dtype=np.float32),
            "g": g,
            "wqkvT": wqkvT,
            "woutT": woutT,
        })
    return in_maps


_NC_CACHE = {}
_RUNNER_CACHE = {}


def _get_nc(debug=False):
    if debug not in _NC_CACHE:
        _NC_CACHE[debug] = build_nc(debug)
    return _NC_CACHE[debug]


def _make_runner(nc, n_cores=N_CORES):
    """Build a persistent jitted SPMD executor (bass2jax multi-core path)."""
    import jax
    from jax.experimental.shard_map import shard_map
    from jax.sharding import Mesh, PartitionSpec
    from concourse.bass2jax import (_bass_exec_p, install_neuronx_cc_hook,
                                    partition_id_tensor)

    install_neuronx_cc_hook()
    partition_name = (nc.partition_id_tensor.name
                      if nc.partition_id_tensor else None)
    in_names, out_names, out_avals, zero_outs = [], [], [], []
    for alloc in nc.m.functions[0].allocations:
        if not isinstance(alloc, mybir.MemoryLocationSet):
            continue
        name = alloc.memorylocations[0].name
        if alloc.kind == "ExternalInput":
            if name != partition_name:
                in_names.append(name)
        elif alloc.kind == "ExternalOutput":
            shape = tuple(alloc.tensor_shape)
            dtype = mybir.dt.np(alloc.dtype)
            out_names.append(name)
            out_avals.append(jax.core.ShapedArray(shape, dtype))
            zero_outs.append(np.zeros(shape, dtype))
    n_params = len(in_names)
    all_names = list(in_names) + out_names
    if partition_name is not None:
        all_names.append(partition_name)

    def _body(*args):
        operands = list(args)
        if partition_name is not None:
            operands.append(partition_id_tensor())
        outs = _bass_exec_p.bind(
            *operands, out_avals=tuple(out_avals), in_names=tuple(all_names),
            out_names=tuple(out_names), lowering_input_output_aliases=(),
            sim_require_finite=True, sim_require_nnan=True, nc=nc)
        return tuple(outs)

    devices = jax.devices()[:n_cores]
    mesh = Mesh(np.asarray(devices), ("core",))
    n_outs = len(out_names)
    sharded = jax.jit(
        shard_map(_body, mesh=mesh,
                  in_specs=(PartitionSpec("core"),) * (n_params + n_outs),
                  out_specs=(PartitionSpec("core"),) * n_outs,
                  check_rep=False),
        keep_unused=True)

    def run(in_maps):
        per_core = [[np.asarray(m[nm]) for nm in in_names] for m in in_maps]
        concat_in = [np.concatenate([per_core[c][i] for c in range(n_cores)], 0)
                     for i in range(n_params)]
        concat_zero = [np.concatenate([z] * n_cores, 0) for z in zero_outs]
        outs = jax.block_until_ready(sharded(*(concat_in + concat_zero)))
        res = []
        for c in range(n_cores):
            m = {}
            for i, nm in enumerate(out_names):
                per = np.asarray(outs[i])
                sh0 = per.shape[0] // n_cores
                m[nm] = per[c * sh0:(c + 1) * sh0]
            res.append(m)
        return res
    return run


def kernel(x, freqs, g, W_qkv, W_out, cu_seqlens=None, max_seqlen=None,
           _debug=False, _trace=False):
    in_maps = _host_prep(np.asarray(x), np.asarray(freqs), np.asarray(g),
                         np.asarray(W_qkv), np.asarray(W_out))
    nc = _get_nc(_debug)
    if _debug not in _RUNNER_CACHE:
        _RUNNER_CACHE[_debug] = _make_runner(nc)
    results = _RUNNER_CACHE[_debug](in_maps)
    out = np.concatenate([results[ci]["out"] for ci in range(N_CORES)], axis=0)
    if _debug:
        return out, results
    return out

